# revision 1
# baseline (speedup 1.0000x reference)
"""LSTMCell (B=65536, H=512) Bass/Tile kernel for 8 trn2 NeuronCores.

Data-parallel over batch: each core processes 8192 rows.
Per 128-row tile:
  z = x + stm                     (DVE)
  zT chunks via PE transpose      (TensorE identity matmul -> PSUM)
  gates = zT.T @ W_packed (f32r)  (TensorE, accumulate over 4 k-chunks)
  pre = gates + bias              (DVE, one [128,2048] op)
  sigmoid(f,i,o) / tanh(g)        (ACT, two ops)
  c = sf + si*tg; h = tanh(c)*so  (DVE + ACT)
"""

import os
import sys

if "/opt/trn_rl_repo" not in sys.path:
    sys.path.insert(0, "/opt/trn_rl_repo")

import numpy as np

import concourse.bacc as bacc
import concourse.mybir as mybir
import concourse.tile as tile

N_CORES = 8
B, H = 65536, 512
B_CORE = B // N_CORES  # 8192
F32 = mybir.dt.float32
F32R = mybir.dt.float32r
AF = mybir.ActivationFunctionType

NEFF_DUMP = "/tmp/lstm_kernel.neff"

# gate order in the packed weight/bias/psum layout: sigmoid gates first so one
# ACT op covers [0:1536], tanh gate last at [1536:2048]
#   slot 0: f (sigmoid), 1: i (sigmoid), 2: o (sigmoid), 3: g (tanh)


def build_module(b_core=B_CORE, n_cores=N_CORES):
    nc = bacc.Bacc(
        "TRN2",
        target_bir_lowering=False,
        debug=False,
        num_devices=n_cores,
    )
    x = nc.dram_tensor("x", [b_core, H], F32, kind="ExternalInput").ap()
    s = nc.dram_tensor("s", [b_core, H], F32, kind="ExternalInput").ap()
    wt = nc.dram_tensor("wt", [128, 8192], F32R, kind="ExternalInput").ap()
    bias = nc.dram_tensor("bias", [128, 2048], F32, kind="ExternalInput").ap()
    ident = nc.dram_tensor("ident", [128, 128], F32, kind="ExternalInput").ap()
    out = nc.dram_tensor("out", [2, b_core, H], F32, kind="ExternalOutput").ap()

    n_tiles = b_core // 128

    with tile.TileContext(nc) as tc:
        with (
            tc.tile_pool(name="const", bufs=1) as cpool,
            tc.tile_pool(name="work", bufs=3) as pool,
            tc.tile_pool(name="pzt", bufs=2, space="PSUM") as pzt,
            tc.tile_pool(name="pgates", bufs=1, space="PSUM") as pg,
        ):
            wt_sb = cpool.tile([128, 8192], F32R)
            nc.sync.dma_start(out=wt_sb[:], in_=wt[:])
            bias_sb = cpool.tile([128, 2048], F32)
            nc.sync.dma_start(out=bias_sb[:], in_=bias[:])
            id_sb = cpool.tile([128, 128], F32)
            nc.sync.dma_start(out=id_sb[:], in_=ident[:])

            for t in range(n_tiles):
                rows = slice(t * 128, (t + 1) * 128)
                x_t = pool.tile([128, H], F32, tag="x")
                nc.sync.dma_start(out=x_t[:], in_=x[rows, :])
                s_t = pool.tile([128, H], F32, tag="s")
                nc.sync.dma_start(out=s_t[:], in_=s[rows, :])
                z_t = pool.tile([128, H], F32, tag="z")
                nc.vector.tensor_add(z_t[:], x_t[:], s_t[:])

                # transpose z into [h_local, b] chunks (PSUM), evacuate to SBUF
                zt_ps = pzt.tile([128, H], F32, tag="ztp")
                for k in range(4):
                    nc.tensor.transpose(
                        zt_ps[:, k * 128 : (k + 1) * 128],
                        z_t[:, k * 128 : (k + 1) * 128],
                        id_sb[:],
                    )
                zt_sb = pool.tile([128, H], F32R, tag="zt")
                nc.vector.tensor_copy(zt_sb[:], zt_ps[:])

                # gates[b, slot*512+j] = sum_h z[b,h] * W_slot[j,h]
                g_ps = pg.tile([128, 2048], F32, tag="gates")
                for k in range(4):
                    lhs = zt_sb[:, k * 128 : (k + 1) * 128]
                    for gs in range(4):
                        nc.tensor.matmul(
                            g_ps[:, gs * 512 : (gs + 1) * 512],
                            lhs,
                            wt_sb[
                                :, gs * 2048 + k * 512 : gs * 2048 + (k + 1) * 512
                            ],
                            start=(k == 0),
                            stop=(k == 3),
                        )

                pre = pool.tile([128, 2048], F32, tag="pre")
                nc.vector.tensor_add(pre[:], g_ps[:], bias_sb[:])
                acts = pool.tile([128, 2048], F32, tag="acts")
                nc.scalar.activation(acts[:, 0:1536], pre[:, 0:1536], AF.Sigmoid)
                nc.scalar.activation(acts[:, 1536:2048], pre[:, 1536:2048], AF.Tanh)

                prod = pool.tile([128, H], F32, tag="prod")
                nc.vector.tensor_mul(prod[:], acts[:, 512:1024], acts[:, 1536:2048])
                c_t = pool.tile([128, H], F32, tag="c")
                nc.vector.tensor_add(c_t[:], acts[:, 0:512], prod[:])
                tc_t = pool.tile([128, H], F32, tag="tc")
                nc.scalar.activation(tc_t[:], c_t[:], AF.Tanh)
                h_t = pool.tile([128, H], F32, tag="h")
                nc.vector.tensor_mul(h_t[:], tc_t[:], acts[:, 1024:1536])

                nc.sync.dma_start(out=out[0, rows, :], in_=c_t[:])
                nc.sync.dma_start(out=out[1, rows, :], in_=h_t[:])

    nc.compile()
    return nc


def round_fp32r(a):
    """Round fp32 to the fp32r grid (11-bit mantissa, RNE)."""
    u = np.ascontiguousarray(a, np.float32).view(np.uint32)
    r = (u.astype(np.uint64) + 0x7FF + ((u >> 12) & 1)) & 0xFFFFF000
    return r.astype(np.uint32).view(np.float32)


def pack_inputs(inputs, short_term_memory, Wf, bf, Wi, bi, Wg, bg, Wo, bo):
    x = np.ascontiguousarray(np.asarray(inputs, np.float32))
    s = np.ascontiguousarray(np.asarray(short_term_memory, np.float32))
    Ws = [Wf, Wi, Wo, Wg]
    bs = [bf, bi, bo, bg]
    wt = np.empty((128, 8192), np.float32)
    for gs, W in enumerate(Ws):
        Wt = np.ascontiguousarray(np.asarray(W, np.float32).T)  # [h, j] = W[j, h]
        # wt[p, gs*2048 + k*512 + j] = W[j, k*128+p]
        wt[:, gs * 2048 : (gs + 1) * 2048] = round_fp32r(
            Wt.reshape(4, 128, 512).transpose(1, 0, 2).reshape(128, 2048)
        )
    bias = np.empty((128, 2048), np.float32)
    for gs, b in enumerate(bs):
        bias[:, gs * 512 : (gs + 1) * 512] = np.asarray(b, np.float32)[None, :]
    ident = np.eye(128, dtype=np.float32)
    return {"x": x, "s": s, "wt": wt, "bias": bias, "ident": ident}


class Runner:
    """Compiles the module once and keeps a reusable jitted executor."""

    def __init__(self, nc=None, n_cores=N_CORES):
        import jax
        from concourse import bass2jax as b2j

        self.jax = jax
        self.n_cores = n_cores
        self.nc = nc or build_module(n_cores=n_cores)
        b2j.install_neuronx_cc_hook()

        # dump the final (renamed) NEFF so neuron-profile can pair it with NTFFs
        if not getattr(b2j, "_neff_dump_patched", False):
            orig = b2j.rename_neff_tensors_and_patch_header

            def _patched(neff_path, mapping):
                data = orig(neff_path, mapping)
                with open(NEFF_DUMP, "wb") as f:
                    f.write(data)
                return data

            b2j.rename_neff_tensors_and_patch_header = _patched
            b2j._neff_dump_patched = True

        from jax.experimental.shard_map import shard_map
        from jax.sharding import Mesh, NamedSharding, PartitionSpec

        part_name = (
            self.nc.partition_id_tensor.name if self.nc.partition_id_tensor else None
        )
        in_names, out_names, out_avals = [], [], []
        self.out_shapes = {}
        for alloc in self.nc.m.functions[0].allocations:
            if not isinstance(alloc, mybir.MemoryLocationSet):
                continue
            name = alloc.memorylocations[0].name
            if alloc.kind == "ExternalInput":
                if name != part_name:
                    in_names.append(name)
            elif alloc.kind == "ExternalOutput":
                out_names.append(name)
                shape = tuple(alloc.tensor_shape)
                dt = mybir.dt.np(alloc.dtype)
                out_avals.append(jax.core.ShapedArray(shape, dt))
                self.out_shapes[name] = (shape, dt)
        self.in_names, self.out_names = in_names, out_names
        nc_ref = self.nc

        bind_names = list(in_names) + list(out_names)
        if part_name is not None:
            bind_names.append(part_name)

        def _body(*args):
            operands = list(args)
            if part_name is not None:
                operands.append(b2j.partition_id_tensor())
            outs = b2j._bass_exec_p.bind(
                *operands,
                out_avals=tuple(out_avals),
                in_names=tuple(bind_names),
                out_names=tuple(out_names),
                lowering_input_output_aliases=(),
                sim_require_finite=False,
                sim_require_nnan=False,
                nc=nc_ref,
            )
            return tuple(outs)

        devices = jax.devices()[: self.n_cores]
        mesh = Mesh(np.asarray(devices), ("core",))
        spec = PartitionSpec("core")
        n_args = len(in_names) + len(out_names)
        self.sharding = NamedSharding(mesh, spec)
        self.fn = jax.jit(
            shard_map(
                _body,
                mesh=mesh,
                in_specs=(spec,) * n_args,
                out_specs=(spec,) * len(out_names),
                check_rep=False,
            ),
            keep_unused=True,
        )
        self._dev_args = None

    def stage(self, packed):
        """Transfer inputs (sharded/replicated as needed) to devices once."""
        jax = self.jax
        nc_n = self.n_cores
        args = []
        for name in self.in_names:
            a = packed[name]
            if name in ("x", "s"):
                glob = a  # already [B, H]; shard axis 0 into 8
            else:
                glob = np.concatenate([a] * nc_n, axis=0)  # replicate
            args.append(glob)
        for name in self.out_names:
            shape, dt = self.out_shapes[name]
            args.append(np.zeros((shape[0] * nc_n,) + shape[1:], dt))
        self._dev_args = [jax.device_put(a, self.sharding) for a in args]

    def execute(self):
        outs = self.fn(*self._dev_args)
        self.jax.block_until_ready(outs)
        return outs

    def run(self, packed):
        self.stage(packed)
        outs = self.execute()
        res = {}
        for name, arr in zip(self.out_names, outs):
            a = np.asarray(arr)  # [n_cores*d0, ...]
            shape, _ = self.out_shapes[name]
            res[name] = a.reshape((self.n_cores, shape[0]) + tuple(shape[1:]))
        return res


_RUNNER = None


def _get_runner():
    global _RUNNER
    if _RUNNER is None:
        _RUNNER = Runner()
    return _RUNNER


def kernel(**inputs):
    r = _get_runner()
    packed = pack_inputs(**inputs)
    res = r.run(packed)
    per_core = res["out"]  # [8, 2, 8192, 512]
    return np.ascontiguousarray(
        per_core.transpose(1, 0, 2, 3).reshape(2, B, H)
    )


if __name__ == "__main__":
    nc = build_module()
    print("module built + compiled OK")



# revision 3
# speedup vs baseline: 2.2509x; 2.2509x over previous
"""LSTMCell (B=65536, H=512) Bass/Tile kernel for 8 trn2 NeuronCores.

Data-parallel over batch: each core processes 8192 rows, as 16 blocks
of 512 batch columns in a TRANSPOSED [feature, batch] layout:

  host: xT/sT staged as bf16 [512, 8192] per core (batch transposed),
        weights packed so each matmul lhsT is a [128h, 128j] W chunk.
  per 512-batch block:
    zT = xT + sT                   (DVE, 4x [128,512] bf16)
    for each of 16 j-chunks (4 gates x 4 slots):
      psum[128j, 512b] = sum_k W_chunk.T @ zT_chunk   (PE, bf16)
      act = sigmoid/tanh(psum + bias_j)  (ACT, per-partition bias fused)
    prod = i*g; c = f + prod; tc = tanh(c); h = tc*o   (DVE + ACT, [128,2048])
    DMA out c,h as bf16 in [h_dim, batch] layout; host transposes back.

vs the previous version this removes the PE transposes, removes the
[128,2048] fp32 DVE bias-add (the old bottleneck: DVE was 76% busy),
and halves DMA traffic (bf16 I/O).
"""

import os
import sys

if "/opt/trn_rl_repo" not in sys.path:
    sys.path.insert(0, "/opt/trn_rl_repo")

import ml_dtypes
import numpy as np

import concourse.bacc as bacc
import concourse.mybir as mybir
import concourse.tile as tile

N_CORES = 8
B, H = 65536, 512
B_CORE = B // N_CORES  # 8192
NB = 512  # batch columns per block
N_BLOCKS = B_CORE // NB  # 16
F32 = mybir.dt.float32
BF16 = mybir.dt.bfloat16
AF = mybir.ActivationFunctionType
NPBF16 = ml_dtypes.bfloat16

NEFF_DUMP = "/tmp/lstm_kernel.neff"

# gate order in the packed weight/bias layout: sigmoid gates first
#   slot 0: f (sigmoid), 1: i (sigmoid), 2: o (sigmoid), 3: g (tanh)


def build_module(b_core=B_CORE, n_cores=N_CORES):
    nc = bacc.Bacc(
        "TRN2",
        target_bir_lowering=False,
        debug=False,
        num_devices=n_cores,
    )
    # xT/sT: [H, b_core] transposed inputs (bf16)
    x = nc.dram_tensor("x", [H, b_core], BF16, kind="ExternalInput").ap()
    s = nc.dram_tensor("s", [H, b_core], BF16, kind="ExternalInput").ap()
    # wt[p, k*2048 + jc*128 + m] = W_gate[jslot*128+m, k*128+p], jc = gate*4+jslot
    wt = nc.dram_tensor("wt", [128, 8192], BF16, kind="ExternalInput").ap()
    # bias[p, jc] = b_gate[jslot*128 + p]
    bias = nc.dram_tensor("bias", [128, 16], F32, kind="ExternalInput").ap()
    # out[0] = c, out[1] = h, both [H, b_core] (transposed; host undoes)
    out = nc.dram_tensor("out", [2, H, b_core], BF16, kind="ExternalOutput").ap()

    with tile.TileContext(nc) as tc:
        with (
            tc.tile_pool(name="const", bufs=1) as cpool,
            tc.tile_pool(name="inp", bufs=3) as ipool,
            tc.tile_pool(name="zp", bufs=2) as zpool,
            tc.tile_pool(name="work", bufs=2) as wpool,
            tc.tile_pool(name="ps", bufs=6, space="PSUM") as pspool,
        ):
            wt_sb = cpool.tile([128, 8192], BF16)
            nc.sync.dma_start(out=wt_sb[:], in_=wt[:])
            bias_sb = cpool.tile([128, 16], F32)
            nc.sync.dma_start(out=bias_sb[:], in_=bias[:])

            for blk in range(N_BLOCKS):
                cols = slice(blk * NB, (blk + 1) * NB)
                z = []
                for k in range(4):
                    rows = slice(k * 128, (k + 1) * 128)
                    x_t = ipool.tile([128, NB], BF16, tag="x")
                    nc.sync.dma_start(out=x_t[:], in_=x[rows, cols])
                    s_t = ipool.tile([128, NB], BF16, tag="s")
                    nc.sync.dma_start(out=s_t[:], in_=s[rows, cols])
                    z_t = zpool.tile([128, NB], BF16, tag=f"z{k}")
                    nc.vector.tensor_add(z_t[:], x_t[:], s_t[:])
                    z.append(z_t)

                # 4 gate tiles, each [128, 4*NB]: slot jslot at cols
                # [jslot*NB:(jslot+1)*NB] holds j = jslot*128 + p
                gt = [
                    wpool.tile([128, 4 * NB], BF16, tag=f"g{g}", name=f"gate{g}")
                    for g in range(4)
                ]
                for g in range(4):
                    func = AF.Sigmoid if g < 3 else AF.Tanh
                    for js in range(4):
                        jc = g * 4 + js
                        ps = pspool.tile([128, NB], F32, tag="ps")
                        for k in range(4):
                            nc.tensor.matmul(
                                ps[:],
                                wt_sb[:, k * 2048 + jc * 128 : k * 2048 + (jc + 1) * 128],
                                z[k][:],
                                start=(k == 0),
                                stop=(k == 3),
                            )
                        nc.scalar.activation(
                            gt[g][:, js * NB : (js + 1) * NB],
                            ps[:],
                            func,
                            bias=bias_sb[:, jc : jc + 1],
                        )

                f_t, i_t, o_t, g_t = gt
                prod = wpool.tile([128, 4 * NB], BF16, tag="prod")
                nc.vector.tensor_mul(prod[:], i_t[:], g_t[:])
                c_t = wpool.tile([128, 4 * NB], BF16, tag="c")
                nc.vector.tensor_add(c_t[:], f_t[:], prod[:])
                tc_t = wpool.tile([128, 4 * NB], BF16, tag="tc")
                nc.scalar.activation(tc_t[:], c_t[:], AF.Tanh)
                h_t = wpool.tile([128, 4 * NB], BF16, tag="h")
                nc.vector.tensor_mul(h_t[:], tc_t[:], o_t[:])

                for js in range(4):
                    hrows = slice(js * 128, (js + 1) * 128)
                    bcols = slice(js * NB, (js + 1) * NB)
                    nc.sync.dma_start(out=out[0, hrows, cols], in_=c_t[:, bcols])
                    nc.sync.dma_start(out=out[1, hrows, cols], in_=h_t[:, bcols])

    nc.compile()
    return nc


def pack_inputs(inputs, short_term_memory, Wf, bf, Wi, bi, Wg, bg, Wo, bo):
    x = np.asarray(inputs, np.float32).astype(NPBF16)
    s = np.asarray(short_term_memory, np.float32).astype(NPBF16)
    # per-core transpose: [B, H] -> [n_cores, H, B_CORE] -> [n_cores*H, B_CORE]
    xT = np.ascontiguousarray(
        x.reshape(N_CORES, B_CORE, H).transpose(0, 2, 1)
    ).reshape(N_CORES * H, B_CORE)
    sT = np.ascontiguousarray(
        s.reshape(N_CORES, B_CORE, H).transpose(0, 2, 1)
    ).reshape(N_CORES * H, B_CORE)

    Ws = [Wf, Wi, Wo, Wg]
    bs = [bf, bi, bo, bg]
    wt = np.empty((128, 8192), NPBF16)
    for k in range(4):
        for g, W in enumerate(Ws):
            # columns [k*2048 + g*512 : +512] = W.T[k*128:(k+1)*128, :]
            wt[:, k * 2048 + g * 512 : k * 2048 + (g + 1) * 512] = (
                np.asarray(W, np.float32).T[k * 128 : (k + 1) * 128, :].astype(NPBF16)
            )
    bias = np.empty((128, 16), np.float32)
    for g, b in enumerate(bs):
        bias[:, g * 4 : (g + 1) * 4] = np.asarray(b, np.float32).reshape(4, 128).T
    return {"x": xT, "s": sT, "wt": wt, "bias": bias}


class Runner:
    """Compiles the module once and keeps a reusable jitted executor."""

    def __init__(self, nc=None, n_cores=N_CORES):
        import jax
        from concourse import bass2jax as b2j

        self.jax = jax
        self.n_cores = n_cores
        self.nc = nc or build_module(n_cores=n_cores)
        b2j.install_neuronx_cc_hook()

        # dump the final (renamed) NEFF so neuron-profile can pair it with NTFFs
        if not getattr(b2j, "_neff_dump_patched", False):
            orig = b2j.rename_neff_tensors_and_patch_header

            def _patched(neff_path, mapping):
                data = orig(neff_path, mapping)
                with open(NEFF_DUMP, "wb") as f:
                    f.write(data)
                return data

            b2j.rename_neff_tensors_and_patch_header = _patched
            b2j._neff_dump_patched = True

        from jax.experimental.shard_map import shard_map
        from jax.sharding import Mesh, NamedSharding, PartitionSpec

        part_name = (
            self.nc.partition_id_tensor.name if self.nc.partition_id_tensor else None
        )
        in_names, out_names, out_avals = [], [], []
        self.out_shapes = {}
        for alloc in self.nc.m.functions[0].allocations:
            if not isinstance(alloc, mybir.MemoryLocationSet):
                continue
            name = alloc.memorylocations[0].name
            if alloc.kind == "ExternalInput":
                if name != part_name:
                    in_names.append(name)
            elif alloc.kind == "ExternalOutput":
                out_names.append(name)
                shape = tuple(alloc.tensor_shape)
                dt = mybir.dt.np(alloc.dtype)
                out_avals.append(jax.core.ShapedArray(shape, dt))
                self.out_shapes[name] = (shape, dt)
        self.in_names, self.out_names = in_names, out_names
        nc_ref = self.nc

        bind_names = list(in_names) + list(out_names)
        if part_name is not None:
            bind_names.append(part_name)

        def _body(*args):
            operands = list(args)
            if part_name is not None:
                operands.append(b2j.partition_id_tensor())
            outs = b2j._bass_exec_p.bind(
                *operands,
                out_avals=tuple(out_avals),
                in_names=tuple(bind_names),
                out_names=tuple(out_names),
                lowering_input_output_aliases=(),
                sim_require_finite=False,
                sim_require_nnan=False,
                nc=nc_ref,
            )
            return tuple(outs)

        devices = jax.devices()[: self.n_cores]
        mesh = Mesh(np.asarray(devices), ("core",))
        spec = PartitionSpec("core")
        n_args = len(in_names) + len(out_names)
        self.sharding = NamedSharding(mesh, spec)
        self.fn = jax.jit(
            shard_map(
                _body,
                mesh=mesh,
                in_specs=(spec,) * n_args,
                out_specs=(spec,) * len(out_names),
                check_rep=False,
            ),
            keep_unused=True,
        )
        self._dev_args = None

    def stage(self, packed):
        """Transfer inputs (sharded/replicated as needed) to devices once."""
        jax = self.jax
        nc_n = self.n_cores
        args = []
        for name in self.in_names:
            a = packed[name]
            if name in ("x", "s"):
                glob = a  # already [n_cores*H, B_CORE]; shard axis 0 into 8
            else:
                glob = np.concatenate([a] * nc_n, axis=0)  # replicate
            args.append(glob)
        for name in self.out_names:
            shape, dt = self.out_shapes[name]
            args.append(np.zeros((shape[0] * nc_n,) + shape[1:], dt))
        self._dev_args = [jax.device_put(a, self.sharding) for a in args]

    def execute(self):
        outs = self.fn(*self._dev_args)
        self.jax.block_until_ready(outs)
        return outs

    def run(self, packed):
        self.stage(packed)
        outs = self.execute()
        res = {}
        for name, arr in zip(self.out_names, outs):
            a = np.asarray(arr)  # [n_cores*d0, ...]
            shape, _ = self.out_shapes[name]
            res[name] = a.reshape((self.n_cores, shape[0]) + tuple(shape[1:]))
        return res


_RUNNER = None


def _get_runner():
    global _RUNNER
    if _RUNNER is None:
        _RUNNER = Runner()
    return _RUNNER


def kernel(**inputs):
    r = _get_runner()
    packed = pack_inputs(**inputs)
    res = r.run(packed)
    per_core = res["out"]  # [8, 2, H, B_CORE] bf16, transposed layout
    full = per_core.transpose(1, 0, 3, 2).reshape(2, B, H)
    return np.ascontiguousarray(full).astype(np.float32)


if __name__ == "__main__":
    nc = build_module()
    print("module built + compiled OK")


# revision 6
# speedup vs baseline: 2.2568x; 1.0026x over previous
"""LSTMCell (B=65536, H=512) Bass/Tile kernel for 8 trn2 NeuronCores.

Data-parallel over batch: each core processes 8192 rows, as 16 blocks
of 512 batch columns in a TRANSPOSED [feature, batch] layout:

  host: xT/sT staged as bf16 [512, 8192] per core (batch transposed),
        weights packed so each matmul lhsT is a [128h, 128j] W chunk.
  per 512-batch block:
    zT = xT + sT                   (DVE, 4x [128,512] bf16)
    for each of 16 j-chunks (4 gates x 4 slots):
      psum[128j, 512b] = sum_k W_chunk.T @ zT_chunk   (PE, bf16)
      act = sigmoid/tanh(psum + bias_j)  (ACT, per-partition bias fused)
    prod = i*g; c = f + prod; tc = tanh(c); h = tc*o   (DVE + ACT, [128,2048])
    DMA out c,h as bf16 in [h_dim, batch] layout; host transposes back.

vs the previous version this removes the PE transposes, removes the
[128,2048] fp32 DVE bias-add (the old bottleneck: DVE was 76% busy),
and halves DMA traffic (bf16 I/O).
"""

import os
import sys

if "/opt/trn_rl_repo" not in sys.path:
    sys.path.insert(0, "/opt/trn_rl_repo")

import ml_dtypes
import numpy as np

import concourse.bacc as bacc
import concourse.mybir as mybir
import concourse.tile as tile

N_CORES = 8
B, H = 65536, 512
B_CORE = B // N_CORES  # 8192
NB = 512  # batch columns per block
N_BLOCKS = B_CORE // NB  # 16
F32 = mybir.dt.float32
BF16 = mybir.dt.bfloat16
AF = mybir.ActivationFunctionType
NPBF16 = ml_dtypes.bfloat16

NEFF_DUMP = "/tmp/lstm_kernel.neff"

# gate order in the packed weight/bias layout — o LAST so the post-matmul
# tail after the final o-matmul is only: o-ACT, h-mul, h-DMA (c's chain
# depends on f/i/g and completes while o matmuls still run)
#   slot 0: f (sigmoid), 1: i (sigmoid), 2: g (tanh), 3: o (sigmoid)


def build_module(b_core=B_CORE, n_cores=N_CORES):
    nc = bacc.Bacc(
        "TRN2",
        target_bir_lowering=False,
        debug=False,
        num_devices=n_cores,
    )
    # xT/sT: [H, b_core] transposed inputs (bf16)
    x = nc.dram_tensor("x", [H, b_core], BF16, kind="ExternalInput").ap()
    s = nc.dram_tensor("s", [H, b_core], BF16, kind="ExternalInput").ap()
    # wt[p, k*2048 + jc*128 + m] = W_gate[jslot*128+m, k*128+p], jc = gate*4+jslot
    wt = nc.dram_tensor("wt", [128, 8192], BF16, kind="ExternalInput").ap()
    # bias[p, jc] = b_gate[jslot*128 + p]
    bias = nc.dram_tensor("bias", [128, 16], F32, kind="ExternalInput").ap()
    # out[0] = c, out[1] = h, both [H, b_core] (transposed; host undoes)
    out = nc.dram_tensor("out", [2, H, b_core], BF16, kind="ExternalOutput").ap()

    with tile.TileContext(nc) as tc:
        with (
            tc.tile_pool(name="const", bufs=1) as cpool,
            tc.tile_pool(name="inp", bufs=3) as ipool,
            tc.tile_pool(name="zp", bufs=2) as zpool,
            tc.tile_pool(name="work", bufs=2) as wpool,
            tc.tile_pool(name="ps", bufs=8, space="PSUM") as pspool,
        ):
            # weights on the gpsimd DGE queue so they stream in parallel
            # with block 0's x/s loads (sync queue); per-k chunks so the
            # first matmul group starts before the full 2MB has landed
            wt_sb = cpool.tile([128, 8192], BF16)
            for k in range(4):
                nc.gpsimd.dma_start(
                    out=wt_sb[:, k * 2048 : (k + 1) * 2048],
                    in_=wt[:, k * 2048 : (k + 1) * 2048],
                )
            bias_sb = cpool.tile([128, 16], F32)
            nc.gpsimd.dma_start(out=bias_sb[:], in_=bias[:])

            for blk in range(N_BLOCKS):
                cols = slice(blk * NB, (blk + 1) * NB)
                z = []
                for k in range(4):
                    rows = slice(k * 128, (k + 1) * 128)
                    x_t = ipool.tile([128, NB], BF16, tag="x")
                    nc.sync.dma_start(out=x_t[:], in_=x[rows, cols])
                    s_t = ipool.tile([128, NB], BF16, tag="s")
                    nc.sync.dma_start(out=s_t[:], in_=s[rows, cols])
                    z_t = zpool.tile([128, NB], BF16, tag=f"z{k}")
                    nc.vector.tensor_add(z_t[:], x_t[:], s_t[:])
                    z.append(z_t)

                # 4 gate tiles, each [128, 4*NB]: slot jslot at cols
                # [jslot*NB:(jslot+1)*NB] holds j = jslot*128 + p
                gt = [
                    wpool.tile([128, 4 * NB], BF16, tag=f"g{g}", name=f"gate{g}")
                    for g in range(4)
                ]
                funcs = [AF.Sigmoid, AF.Sigmoid, AF.Tanh, AF.Sigmoid]
                f_t, i_t, g_t, o_t = gt

                def gate_group(g, js):
                    jc = g * 4 + js
                    ps = pspool.tile([128, NB], F32, tag="ps", name="ps")
                    for k in range(4):
                        nc.tensor.matmul(
                            ps[:],
                            wt_sb[:, k * 2048 + jc * 128 : k * 2048 + (jc + 1) * 128],
                            z[k][:],
                            start=(k == 0),
                            stop=(k == 3),
                        )
                    nc.scalar.activation(
                        gt[g][:, js * NB : (js + 1) * NB],
                        ps[:],
                        funcs[g],
                        bias=bias_sb[:, jc : jc + 1],
                    )

                for g in range(3):  # f, i, g gates
                    for js in range(4):
                        gate_group(g, js)

                # c-chain: runs on DVE/ACT while the o-gate matmuls proceed
                prod = wpool.tile([128, 4 * NB], BF16, tag="prod")
                nc.vector.tensor_mul(prod[:], i_t[:], g_t[:])
                c_t = wpool.tile([128, 4 * NB], BF16, tag="c")
                nc.vector.tensor_add(c_t[:], f_t[:], prod[:])
                tc_t = wpool.tile([128, 4 * NB], BF16, tag="tc")
                nc.scalar.activation(tc_t[:], c_t[:], AF.Tanh)
                for js in range(4):
                    hrows = slice(js * 128, (js + 1) * 128)
                    nc.gpsimd.dma_start(
                        out=out[0, hrows, cols], in_=c_t[:, js * NB : (js + 1) * NB]
                    )

                # o gate + per-slot h so the post-matmul tail is one slot deep
                h_t = wpool.tile([128, 4 * NB], BF16, tag="h")
                for js in range(4):
                    gate_group(3, js)
                    bcols = slice(js * NB, (js + 1) * NB)
                    nc.vector.tensor_mul(h_t[:, bcols], tc_t[:, bcols], o_t[:, bcols])
                    hrows = slice(js * 128, (js + 1) * 128)
                    nc.gpsimd.dma_start(out=out[1, hrows, cols], in_=h_t[:, bcols])

    nc.compile()
    return nc


def pack_inputs(inputs, short_term_memory, Wf, bf, Wi, bi, Wg, bg, Wo, bo):
    x = np.asarray(inputs, np.float32).astype(NPBF16)
    s = np.asarray(short_term_memory, np.float32).astype(NPBF16)
    # per-core transpose: [B, H] -> [n_cores, H, B_CORE] -> [n_cores*H, B_CORE]
    xT = np.ascontiguousarray(
        x.reshape(N_CORES, B_CORE, H).transpose(0, 2, 1)
    ).reshape(N_CORES * H, B_CORE)
    sT = np.ascontiguousarray(
        s.reshape(N_CORES, B_CORE, H).transpose(0, 2, 1)
    ).reshape(N_CORES * H, B_CORE)

    Ws = [Wf, Wi, Wg, Wo]
    bs = [bf, bi, bg, bo]
    wt = np.empty((128, 8192), NPBF16)
    for k in range(4):
        for g, W in enumerate(Ws):
            # columns [k*2048 + g*512 : +512] = W.T[k*128:(k+1)*128, :]
            wt[:, k * 2048 + g * 512 : k * 2048 + (g + 1) * 512] = (
                np.asarray(W, np.float32).T[k * 128 : (k + 1) * 128, :].astype(NPBF16)
            )
    bias = np.empty((128, 16), np.float32)
    for g, b in enumerate(bs):
        bias[:, g * 4 : (g + 1) * 4] = np.asarray(b, np.float32).reshape(4, 128).T
    return {"x": xT, "s": sT, "wt": wt, "bias": bias}


class Runner:
    """Compiles the module once and keeps a reusable jitted executor."""

    def __init__(self, nc=None, n_cores=N_CORES):
        import jax
        from concourse import bass2jax as b2j

        self.jax = jax
        self.n_cores = n_cores
        self.nc = nc or build_module(n_cores=n_cores)
        b2j.install_neuronx_cc_hook()

        # dump the final (renamed) NEFF so neuron-profile can pair it with NTFFs
        if not getattr(b2j, "_neff_dump_patched", False):
            orig = b2j.rename_neff_tensors_and_patch_header

            def _patched(neff_path, mapping):
                data = orig(neff_path, mapping)
                with open(NEFF_DUMP, "wb") as f:
                    f.write(data)
                return data

            b2j.rename_neff_tensors_and_patch_header = _patched
            b2j._neff_dump_patched = True

        from jax.experimental.shard_map import shard_map
        from jax.sharding import Mesh, NamedSharding, PartitionSpec

        part_name = (
            self.nc.partition_id_tensor.name if self.nc.partition_id_tensor else None
        )
        in_names, out_names, out_avals = [], [], []
        self.out_shapes = {}
        for alloc in self.nc.m.functions[0].allocations:
            if not isinstance(alloc, mybir.MemoryLocationSet):
                continue
            name = alloc.memorylocations[0].name
            if alloc.kind == "ExternalInput":
                if name != part_name:
                    in_names.append(name)
            elif alloc.kind == "ExternalOutput":
                out_names.append(name)
                shape = tuple(alloc.tensor_shape)
                dt = mybir.dt.np(alloc.dtype)
                out_avals.append(jax.core.ShapedArray(shape, dt))
                self.out_shapes[name] = (shape, dt)
        self.in_names, self.out_names = in_names, out_names
        nc_ref = self.nc

        bind_names = list(in_names) + list(out_names)
        if part_name is not None:
            bind_names.append(part_name)

        def _body(*args):
            operands = list(args)
            if part_name is not None:
                operands.append(b2j.partition_id_tensor())
            outs = b2j._bass_exec_p.bind(
                *operands,
                out_avals=tuple(out_avals),
                in_names=tuple(bind_names),
                out_names=tuple(out_names),
                lowering_input_output_aliases=(),
                sim_require_finite=False,
                sim_require_nnan=False,
                nc=nc_ref,
            )
            return tuple(outs)

        devices = jax.devices()[: self.n_cores]
        mesh = Mesh(np.asarray(devices), ("core",))
        spec = PartitionSpec("core")
        n_args = len(in_names) + len(out_names)
        self.sharding = NamedSharding(mesh, spec)
        self.fn = jax.jit(
            shard_map(
                _body,
                mesh=mesh,
                in_specs=(spec,) * n_args,
                out_specs=(spec,) * len(out_names),
                check_rep=False,
            ),
            keep_unused=True,
        )
        self._dev_args = None

    def stage(self, packed):
        """Transfer inputs (sharded/replicated as needed) to devices once."""
        jax = self.jax
        nc_n = self.n_cores
        args = []
        for name in self.in_names:
            a = packed[name]
            if name in ("x", "s"):
                glob = a  # already [n_cores*H, B_CORE]; shard axis 0 into 8
            else:
                glob = np.concatenate([a] * nc_n, axis=0)  # replicate
            args.append(glob)
        for name in self.out_names:
            shape, dt = self.out_shapes[name]
            args.append(np.zeros((shape[0] * nc_n,) + shape[1:], dt))
        self._dev_args = [jax.device_put(a, self.sharding) for a in args]

    def execute(self):
        outs = self.fn(*self._dev_args)
        self.jax.block_until_ready(outs)
        return outs

    def run(self, packed):
        self.stage(packed)
        outs = self.execute()
        res = {}
        for name, arr in zip(self.out_names, outs):
            a = np.asarray(arr)  # [n_cores*d0, ...]
            shape, _ = self.out_shapes[name]
            res[name] = a.reshape((self.n_cores, shape[0]) + tuple(shape[1:]))
        return res


_RUNNER = None


def _get_runner():
    global _RUNNER
    if _RUNNER is None:
        _RUNNER = Runner()
    return _RUNNER


def kernel(**inputs):
    r = _get_runner()
    packed = pack_inputs(**inputs)
    res = r.run(packed)
    per_core = res["out"]  # [8, 2, H, B_CORE] bf16, transposed layout
    full = per_core.transpose(1, 0, 3, 2).reshape(2, B, H)
    return np.ascontiguousarray(full).astype(np.float32)


if __name__ == "__main__":
    nc = build_module()
    print("module built + compiled OK")
